# revision 20
# baseline (speedup 1.0000x reference)
"""Distributed multi-head attention kernel for one TRN2 chip (8 NeuronCores).

Problem: x[2, 2048, 1024] -> fused QKV proj (16 heads x 64) -> softmax attention
-> output proj, weights packed as in the reference (qkv interleaved [3, h, d]).

Sharding: 2-way data parallel on batch x 4-way tensor parallel on heads.
Core c = (b = c // 4, g = c % 4) gets batch b and heads [4g, 4g+4).

v4 design:
  - x^T via PE transposes (the DMA-transpose XBAR is a serial shared resource
    at ~1.2us/64KB -- measured, far too slow for 4MB).
  - scores in bf16 exactly like the well-tuned baseline (fp8 DoubleRow cannot
    help a 64-deep contraction: the PE streams 2 fp8 elements/cycle, so the
    win exists only when both subtiles carry real contraction data).
  - V' is ones-augmented on the host (zero W column + bias 1.0 per head,
    65 cols/head): the PV matmul's 65th output row accumulates sum(exp) --
    the softmax denominator -- for free in PSUM.  This kills the DVE
    running-sum (~90us) and the dn matmuls of the baseline.
  - PV runs fp8 DoubleRow over k-chunk PAIRS (contraction 256 = 2x128):
    exp writes fp8 e directly; V stored fp8 [128, 2, 320] (80-col head
    stride for the 16B alignment dual-fp8 LDWEIGHTS requires).  2x fewer
    PV matmul instructions streaming at 2 elem/cycle.
  - normalization: dn row (psum partition 64) is DMA'd to partition 0,
    reciprocal + gpsimd partition-broadcast + DVE muls; head1's 64 rows is
    DMA-stacked into partitions 64:128 of the pair's oT tile (DoubleRow
    outputs must start at partition 0).
  - output projection bf16 with b_out/4 folded pre-collective; one bf16
    ReduceScatter per 512-row block (collectives cost ~8us each on the
    serial CC core -- finer granularity loses).
  - background PE work (x-chunk transposes, K rb1-3, V chunks, next-block Q,
    prev-block outproj) is interleaved into the score/exp/PV loop in an
    order that respects the in-order PE queue.
"""
import numpy as np
import ml_dtypes

from concourse import mybir, tile, bacc
from concourse.bass_utils import run_bass_kernel_spmd

S = 2048       # sequence length (one batch element per core)
D = 1024       # embed dim
HL = 4         # local heads per core
HD = 64        # head dim
VW = 65        # V' cols per head (64 + ones column)
VP = 80        # padded per-head stride in v2 tiles (16B-aligned fp8)
QKVC = 512 + HL * VW   # 772 local qkv columns (Q 256 | K 256 | V' 260)
VOFF = 512     # V column offset within the shard
BLK = 512      # s_q / s_k block size
NBLK = S // BLK      # 4
KC = S // 128        # 16 s_k chunks
DC = D // 128        # 8 dmodel chunks
F32 = mybir.dt.float32
BF16 = mybir.dt.bfloat16
FP8 = mybir.dt.float8e4
EXP = mybir.ActivationFunctionType.Exp
SCALE = 1.0 / np.sqrt(HD)
DR = mybir.MatmulPerfMode.DoubleRow

REPLICA_GROUPS = [[0, 1, 2, 3], [4, 5, 6, 7]]


def build_nc(debug_taps=False):
    from contextlib import ExitStack

    nc = bacc.Bacc("TRN2", target_bir_lowering=False, debug=False, num_devices=8)
    x_ext = nc.declare_dram_parameter("x", [S, D], BF16, isOutput=False)
    wqkv_ext = nc.declare_dram_parameter("wqkv", [D, QKVC], BF16, isOutput=False)
    bqkv_ext = nc.declare_dram_parameter("bqkv", [QKVC], F32, isOutput=False)
    wout_ext = nc.declare_dram_parameter("wout", [HL * HD, D], BF16, isOutput=False)
    bout_ext = nc.declare_dram_parameter("bout", [D], F32, isOutput=False)
    out_ext = nc.declare_dram_parameter("out", [NBLK * 128, D], BF16, isOutput=True)
    if debug_taps:
        dbg = {
            "kT0": nc.declare_dram_parameter("d_kT0", [128, S], BF16, isOutput=True),
            "qT00": nc.declare_dram_parameter("d_qT00", [128, BLK], BF16, isOutput=True),
            "v2_0": nc.declare_dram_parameter("d_v2_0", [128, 2 * HL * VP], FP8, isOutput=True),
            "e2_0": nc.declare_dram_parameter("d_e2_0", [128, 2 * 2 * BLK], FP8, isOutput=True),
            "pv0": nc.declare_dram_parameter("d_pv0", [128, BLK], F32, isOutput=True),
            "pv1": nc.declare_dram_parameter("d_pv1", [128, BLK], F32, isOutput=True),
            "rc64": nc.declare_dram_parameter("d_rc64", [128, 2 * BLK], F32, isOutput=True),
            "rbt0": nc.declare_dram_parameter("d_rbt0", [64, BLK], F32, isOutput=True),
            "rbt1": nc.declare_dram_parameter("d_rbt1", [64, BLK], F32, isOutput=True),
            "ot0": nc.declare_dram_parameter("d_ot0", [128, BLK], BF16, isOutput=True),
        }

    with tile.TileContext(nc) as tc, ExitStack() as top:
        const = top.enter_context(tc.tile_pool(name="const", bufs=1))
        xT_pool = top.enter_context(tc.tile_pool(name="xT", bufs=1))
        xstage = top.enter_context(tc.tile_pool(name="xstage", bufs=KC))
        wq_pool = top.enter_context(tc.tile_pool(name="wq", bufs=DC))
        woutp = top.enter_context(tc.tile_pool(name="woutp", bufs=2))
        kT_pool = top.enter_context(tc.tile_pool(name="kT", bufs=2))
        qT_pool = top.enter_context(tc.tile_pool(name="qT", bufs=2 * NBLK))
        v_pool = top.enter_context(tc.tile_pool(name="v", bufs=KC // 2))
        e_pool = top.enter_context(tc.tile_pool(name="e", bufs=3))
        ot_pool = top.enter_context(tc.tile_pool(name="ot", bufs=4))
        rc_pool = top.enter_context(tc.tile_pool(name="rcp", bufs=2))
        rb_pool = top.enter_context(tc.tile_pool(name="rbp", bufs=2))
        ott_pool = top.enter_context(tc.tile_pool(name="ottp", bufs=2))
        stage = top.enter_context(tc.tile_pool(name="stage", bufs=6))
        rs_dram = top.enter_context(tc.tile_pool(name="rs_dram", bufs=4,
                                                 space="DRAM"))
        big_ps = top.enter_context(tc.tile_pool(name="big_ps", bufs=3,
                                                space="PSUM"))
        pv_ps = top.enter_context(tc.tile_pool(name="pv_ps", bufs=2, space="PSUM"))

        XQ = [nc.gpsimd, nc.sync, nc.scalar]

        # ---- bias rows first (sync), then x chunks 0-3, wq, x 4-15 ----
        bqk_sb = const.tile([128, 4], F32)        # per-partition q/k bias, col m
        nc.sync.dma_start(out=bqk_sb[:, :],
                          in_=bqkv_ext[0:512].rearrange("(m p) -> p m", p=128))
        bv_row = const.tile([1, HL * VW], F32)
        nc.sync.dma_start(out=bv_row[:, :], in_=bqkv_ext[VOFF:QKVC][None, :])
        bout_f = const.tile([1, D], F32)
        nc.sync.dma_start(out=bout_f[:, :], in_=bout_ext[None, :])

        xf_tiles = {}

        def prefetch_x(sc):
            xf = xstage.tile([128, D], BF16, tag="x_bf", name="x_bf")
            XQ[sc % 3].dma_start(out=xf[:, :],
                                 in_=x_ext[sc * 128:(sc + 1) * 128, :])
            xf_tiles[sc] = xf

        for sc in range(4):
            prefetch_x(sc)
        wq_bf = []
        for c in range(DC):
            wb = wq_pool.tile([128, QKVC], BF16, tag="wq_bf", name="wq_bf")
            XQ[(c + 1) % 3].dma_start(out=wb[:, :],
                                      in_=wqkv_ext[c * 128:(c + 1) * 128, :])
            wq_bf.append(wb)
        for sc in range(4, KC):
            prefetch_x(sc)

        ident = const.tile([128, 128], BF16)
        from concourse.masks import make_identity
        make_identity(nc, ident[:, :])

        bv_sb = const.tile([128, HL * VW], F32)
        nc.gpsimd.partition_broadcast(bv_sb[:, :], bv_row[:, :])
        bout_q = const.tile([128, D], F32)        # b_out / n_group, pre-RS
        nc.gpsimd.partition_broadcast(bout_q[:, :], bout_f[:, :])
        nc.vector.tensor_scalar_mul(bout_q[:, :], bout_q[:, :], 0.25)

        # ---- persistent attention operand tiles ----
        xT_all = xT_pool.tile([128, DC * S], BF16, tag="xT", name="xT")
        xT = [xT_all[:, c * S:(c + 1) * S] for c in range(DC)]
        xT3 = xT_all[:, :].rearrange("p (c s) -> p c s", s=S)
        kT = [kT_pool.tile([128, S], BF16, tag="kT", name="kT") for _ in range(2)]
        qT = [[qT_pool.tile([128, BLK], BF16, tag="qT", name="qT")
               for _ in range(NBLK)] for _ in range(2)]
        # v2[j]: chunk pair j -> [128 k, 2 (kc parity), 4h x 80] fp8
        v2 = [v_pool.tile([128, 2, HL * VP], FP8, tag="v2", name="v2")
              for _ in range(KC // 2)]

        wout_bf = []
        for hp in range(2):
            wb = woutp.tile([128, D], BF16, tag="wout_bf")
            nc.scalar.dma_start(out=wb[:, :],
                                in_=wout_ext[hp * 128:(hp + 1) * 128, :])
            wout_bf.append(wb)

        # ---- PE-work helpers, interleaved as background closures ----
        def transpose_chunk(sc):
            xf = xf_tiles.pop(sc)
            for ch in range(2):       # 4 transposes share one psum
                tp4 = big_ps.tile([128, 512], BF16, tag="sp", name="tp")
                for cc in range(4):
                    c = ch * 4 + cc
                    nc.tensor.transpose(tp4[:, cc * 128:(cc + 1) * 128],
                                        xf[:, c * 128:(c + 1) * 128],
                                        ident[:, :])
                nc.vector.tensor_copy(
                    xT3[:, ch * 4:(ch + 1) * 4, sc * 128:(sc + 1) * 128],
                    tp4[:, :].rearrange("p (c s) -> p c s", s=128))

        def k_proj(p, rb):
            ps = big_ps.tile([128, BLK], F32, tag="sp", name="kproj")
            for c in range(DC):
                nc.tensor.matmul(ps[:, :],
                                 wq_bf[c][:, 256 + p * 128:256 + (p + 1) * 128],
                                 xT[c][:, rb * BLK:(rb + 1) * BLK],
                                 start=(c == 0), stop=(c == DC - 1))
            nc.vector.tensor_add(kT[p][:, rb * BLK:(rb + 1) * BLK], ps[:, :],
                                 bqk_sb[:, 2 + p:3 + p].to_broadcast((128, BLK)))

        def q_proj(p, blk):
            ps = big_ps.tile([128, BLK], F32, tag="sp", name="qproj")
            for c in range(DC):
                nc.tensor.matmul(ps[:, :], wq_bf[c][:, p * 128:(p + 1) * 128],
                                 xT[c][:, blk * BLK:(blk + 1) * BLK],
                                 start=(c == 0), stop=(c == DC - 1))
            nc.vector.tensor_add(qT[p][blk][:, :], ps[:, :],
                                 bqk_sb[:, p:p + 1].to_broadcast((128, BLK)))

        def v_proj(sc):
            ps = big_ps.tile([128, HL * VW], F32, tag="sp", name="vproj")
            for c in range(DC):
                nc.tensor.matmul(ps[:, :], xT[c][:, sc * 128:(sc + 1) * 128],
                                 wq_bf[c][:, VOFF:QKVC],
                                 start=(c == 0), stop=(c == DC - 1))
            nc.vector.tensor_add(
                v2[sc // 2][:, sc % 2, :].rearrange("p (h c) -> p h c", c=VP)
                [:, :, 0:VW],
                ps[:, :].rearrange("p (h c) -> p h c", c=VW),
                bv_sb[:, :].rearrange("p (h c) -> p h c", c=VW))

        # ---- output projection + per-block ReduceScatter ----
        def outproj_sq(ot_pair, rs_in, sq):
            po = big_ps.tile([128, D], F32, tag="sp", name="po")
            qs = slice(sq * 128, (sq + 1) * 128)
            for hp in range(2):
                for nh in range(2):
                    ns = slice(nh * 512, (nh + 1) * 512)
                    nc.tensor.matmul(po[:, ns], ot_pair[hp][:, qs],
                                     wout_bf[hp][:, ns],
                                     start=(hp == 0), stop=(hp == 1),
                                     skip_group_check=True)
            st = stage.tile([128, D], BF16, tag="st", name="st")
            nc.vector.tensor_add(st[:, :], po[:, :], bout_q[:, :])
            nc.gpsimd.dma_start(out=rs_in[sq * 128:(sq + 1) * 128, :], in_=st[:, :])

        def emit_rs(pblk, rs_in):
            rs_out = rs_dram.tile([128, D], BF16, tag="rs_out", name="rs_out")
            nc.gpsimd.collective_compute(
                "ReduceScatter", mybir.AluOpType.add,
                replica_groups=REPLICA_GROUPS,
                ins=[rs_in[:, :].opt()], outs=[rs_out[:, :].opt()])
            nc.sync.dma_start(out=out_ext[pblk * 128:(pblk + 1) * 128, :],
                              in_=rs_out[:, :])

        # ---- ramp-critical: transposes sc0-3, K rb0, Q blk0 ----
        for sc in range(4):
            transpose_chunk(sc)
        for p in range(2):
            k_proj(p, 0)
        for p in range(2):
            q_proj(p, 0)

        # background PE task queue for blk0-p0: everything else blk0 needs,
        # ordered so PE-queue dependencies stay acyclic under pump=3/t
        # (deadlines: K rb emitted before t=4rb, V pair j's two chunks
        # before t=2j+3, transposes before the K rb that reads them).
        T = transpose_chunk
        K = k_proj
        V = v_proj
        bg0 = [lambda: V(0), lambda: V(1),
               lambda: T(4), lambda: T(5), lambda: T(6), lambda: T(7),
               lambda: K(0, 1), lambda: K(1, 1),
               lambda: V(2), lambda: V(3), lambda: V(4), lambda: V(5),
               lambda: T(8), lambda: T(9), lambda: T(10), lambda: T(11),
               lambda: K(0, 2), lambda: K(1, 2),
               lambda: V(6), lambda: V(7),
               lambda: T(12), lambda: T(13), lambda: T(14), lambda: T(15),
               lambda: K(0, 3), lambda: K(1, 3)] + \
              [lambda sc=sc: V(sc) for sc in range(8, KC)]

        def attention_pair(blk, p, bg, bgn, prev):
            pv = [pv_ps.tile([128, BLK], F32, tag="pv", name="pv")
                  for _ in range(2)]
            e2s = {}
            bgi = 0
            for t in range(KC + 3):
                if t < KC:
                    sp = big_ps.tile([128, 2 * BLK], F32, tag="sp", name="sp")
                    ks = slice(t * 128, (t + 1) * 128)
                    nc.tensor.matmul(sp[:, 0:BLK],
                                     kT[p][0:64, ks], qT[p][blk][0:64, :],
                                     start=True, stop=True)
                    nc.tensor.matmul(sp[:, BLK:],
                                     kT[p][64:128, ks], qT[p][blk][64:128, :],
                                     start=True, stop=True)
                    if t % 2 == 0:
                        e2s[t // 2] = e_pool.tile([128, 2, 2 * BLK], FP8,
                                                  tag="e2", name="e2")
                    nc.scalar.activation(e2s[t // 2][:, t % 2, :], sp[:, :],
                                         EXP, scale=float(SCALE))
                if t >= 3 and (t - 3) % 2 == 0 and (t - 3) // 2 < KC // 2:
                    j = (t - 3) // 2
                    e2 = e2s[j]
                    for hh in range(2):
                        h = 2 * p + hh
                        nc.tensor.matmul(
                            pv[hh][0:VP, :],
                            v2[j][:, :, h * VP:(h + 1) * VP],
                            e2[:, :, hh * BLK:(hh + 1) * BLK],
                            start=(j == 0), stop=(j == KC // 2 - 1),
                            perf_mode=DR, skip_group_check=True)
                    if j >= 1:
                        del e2s[j - 1]
                    if debug_taps and blk == 0 and p == 0 and j == 0:
                        nc.sync.dma_start(out=dbg["e2_0"][:, :],
                                          in_=e2[:, :].rearrange("p a b -> p (a b)"))
                for _ in range(bgn):
                    if bgi < len(bg):
                        bg[bgi]()
                        bgi += 1
                if prev is not None:
                    if t in (2, 5, 8, 11):
                        outproj_sq(prev[0], prev[1], (t - 2) // 3)
                    elif t == 14:
                        emit_rs(prev[2], prev[1])
                        prev = None
            while bgi < len(bg):
                bg[bgi]()
                bgi += 1

            # ---- softmax normalize: dn sits in psum partition 64.
            # reciprocal partition-aligned at 64 (psum -> sbuf), tiny DMA to
            # partition 0, then partition_broadcast (which reads partition 0).
            rc64 = rc_pool.tile([128, 2 * BLK], F32, tag="rc64", name="rc64")
            rc2 = rc_pool.tile([128, 2 * BLK], F32, tag="rc64", name="rc2")
            ot_p = ot_pool.tile([128, BLK], BF16, tag="ot", name="ot")
            ot_tmp = ott_pool.tile([64, BLK], BF16, tag="ott", name="ott")
            for hh in range(2):
                cs = slice(hh * BLK, (hh + 1) * BLK)
                # partition-aligned psum->sbuf copy of the dn row, DMA it to
                # partition 0, reciprocal there (DVE reciprocal misreads PSUM
                # at partition offsets; tensor ops handle it fine).
                nc.vector.tensor_copy(rc64[64:65, cs], pv[hh][64:65, :])
                nc.gpsimd.dma_start(out=rc64[0:1, cs], in_=rc64[64:65, cs])
                nc.vector.reciprocal_approx_fast(rc2[0:1, cs], rc64[0:1, cs])
                rbt = rb_pool.tile([64, BLK], F32, tag="rb", name="rb")
                nc.gpsimd.partition_broadcast(rbt[:, :], rc2[0:1, cs])
                if debug_taps and blk == 0 and p == 0:
                    pvc = stage.tile([128, BLK], F32, tag="dbgpv", name="dbgpv")
                    nc.vector.tensor_copy(pvc[:, :], pv[hh][:, :])
                    nc.sync.dma_start(out=dbg["pv%d" % hh][:, :], in_=pvc[:, :])
                    nc.sync.dma_start(out=dbg["rbt%d" % hh][:, :], in_=rbt[:, :])
                if hh == 0:
                    nc.vector.tensor_mul(ot_p[0:64, :], pv[hh][0:64, :],
                                         rbt[:, :])
                else:
                    nc.vector.tensor_mul(ot_tmp[:, :], pv[hh][0:64, :],
                                         rbt[:, :])
                    nc.gpsimd.dma_start(out=ot_p[64:128, :], in_=ot_tmp[:, :])
            if debug_taps and blk == 0 and p == 0:
                nc.sync.dma_start(out=dbg["rc64"][:, :], in_=rc64[:, :])
                nc.sync.dma_start(out=dbg["ot0"][:, :], in_=ot_p[:, :])
                nc.sync.dma_start(out=dbg["kT0"][:, :], in_=kT[0][:, :])
                nc.sync.dma_start(out=dbg["qT00"][:, :], in_=qT[0][0][:, :])
                nc.sync.dma_start(out=dbg["v2_0"][:, :],
                                  in_=v2[0][:, :].rearrange("p a b -> p (a b)"))
            return ot_p

        prev = None   # (ot pair tiles, rs_in, block index)
        for blk in range(NBLK):
            ots = []
            for p in range(2):
                if blk == 0 and p == 0:
                    bg, bgn = bg0, 3
                elif blk < NBLK - 1 and p == 1:
                    bg = [lambda p_=p_, b_=blk + 1: q_proj(p_, b_)
                          for p_ in range(2)]
                    bgn = 1
                else:
                    bg, bgn = [], 0
                ots.append(attention_pair(blk, p, bg, bgn,
                                          prev if p == 0 else None))
                if p == 0:
                    prev = None
            rs_in = rs_dram.tile([BLK, D], BF16, tag="rs_in", name="rs_in")
            prev = (ots, rs_in, blk)

        # drain the last block's output projection + ReduceScatter
        for sq in range(4):
            outproj_sq(prev[0], prev[1], sq)
        emit_rs(prev[2], prev[1])

    nc.compile()
    return nc


_NC = None


def make_in_maps(x, W_qkv, b_qkv, W_out, b_out):
    bf = ml_dtypes.bfloat16
    in_maps = []
    for c in range(8):
        b, g = c // 4, c % 4
        heads = range(4 * g, 4 * g + 4)
        wq = np.zeros((D, QKVC), np.float32)
        bq = np.zeros((QKVC,), np.float32)
        for t in range(2):  # Q, K
            cols = np.concatenate(
                [np.arange(t * 1024 + h * 64, t * 1024 + h * 64 + 64)
                 for h in heads])
            wq[:, t * 256:(t + 1) * 256] = W_qkv[:, cols]
            bq[t * 256:(t + 1) * 256] = b_qkv[cols]
        for i, h in enumerate(heads):  # V with ones column per head
            vc = np.arange(2048 + h * 64, 2048 + h * 64 + 64)
            wq[:, VOFF + i * VW:VOFF + i * VW + 64] = W_qkv[:, vc]
            bq[VOFF + i * VW:VOFF + i * VW + 64] = b_qkv[vc]
            bq[VOFF + i * VW + 64] = 1.0
        in_maps.append({
            "x": np.ascontiguousarray(x[b]).astype(bf),
            "wqkv": wq.astype(bf),
            "bqkv": bq,
            "wout": np.ascontiguousarray(
                W_out[g * 256:(g + 1) * 256, :]).astype(bf),
            "bout": np.ascontiguousarray(b_out),
        })
    return in_maps


def kernel(x, W_qkv, b_qkv, W_out, b_out):
    global _NC
    if _NC is None:
        _NC = build_nc()

    in_maps = make_in_maps(x, W_qkv, b_qkv, W_out, b_out)
    res = run_bass_kernel_spmd(_NC, in_maps, core_ids=list(range(8)))

    # core (b, g), local row r = blk*128 + j  <->  full row = blk*512 + g*128 + j
    out = np.empty((2, S, D), np.float32)
    for c in range(8):
        b, g = c // 4, c % 4
        r = np.asarray(res.results[c]["out"]).astype(np.float32)
        for k in range(NBLK):
            out[b, k * BLK + g * 128: k * BLK + (g + 1) * 128, :] = \
                r[k * 128:(k + 1) * 128, :]
    return out
